# revision 29
# baseline (speedup 1.0000x reference)
"""Trainium2 Bass kernel for nn_CrossRPEAttention (B=4, H=12, DIM=768, Q=577, N=2305).

Sharding: 8 cores = batch(4) x head-half(2). Each core computes, for its
(b, hh): k/v projections for its 384 channels, cross-attention with iRPE
contextual bias for 6 heads, and a partial output projection. Host sums the
two head-half partials per batch and adds proj_b.

Attention is computed in S^T layout (keys on partitions, queries on free dim)
so no on-device transposes are needed anywhere:
  - logits^T tile per key-tile via PE (contraction HD=64)
  - softmax shift-invariance: bias is taken relative to the far-field bucket
    (u=3), which covers all rounded-distances >= 4, so the remaining bias
    correction is multiplicative and local: |dy|,|dx| <= 3 (band +-75 in
    raveled index)
  - unnorm = exp(scale*S^T) * (1 + sum_u F'_u[i] * M_u[js,i]) applied as
    banded rank-full masks on DVE, folded into extra accumulating matmuls
  - denominator comes free from a ones-column appended to v
  - the cls-query column (i=0) bias is a per-column constant -> cancels in
    softmax and is skipped entirely
"""
import sys
import numpy as np

sys.path.insert(0, "/opt/trn_rl_repo")

import concourse.bass as bass
import concourse.bacc as bacc
import concourse.mybir as mybir
import concourse.tile as tile
from concourse.bass_utils import run_bass_kernel_spmd

try:
    import ml_dtypes
    BF16_NP = ml_dtypes.bfloat16
except ImportError:  # pragma: no cover
    BF16_NP = np.float32

F32 = mybir.dt.float32
BF16 = mybir.dt.bfloat16
OP = mybir.AluOpType
AF = mybir.ActivationFunctionType

# ---------------- problem constants ----------------
B, DIM, H, HD, GRID = 4, 768, 12, 64, 24
P = GRID * GRID            # 576 spatial patches
NM = 4                     # modalities
Q = 1 + P                  # 577 queries
NKEY = 1 + NM * P          # 2305 keys
SCALE = HD ** -0.5
ALPHA, BETA, GAMMA = 1.9, 3.8, 15.2
BAND = 75
JT_STARTS = [0, 128, 256, 384, 512]
JT_SIZES = [128, 128, 128, 128, 64]
N_LOCAL = 3                # local correction buckets u = 0,1,2
BASE_BUCKET = 3            # far-field bucket (rounded dist >= 4)
CLS_BUCKET = 5
NH = 6                     # heads per core
CH = NH * HD               # 384 channels per core
IB = [(0, 512), (512, Q)]  # query (free dim) bank splits

# attention-path dtype (flip to F32/np.float32 if precision demands)
DT_E = BF16
DT_E_NP = BF16_NP
DEBUG_TAPS = False


# ---------------- host-side constants ----------------
def _sincos_1d(d, pos):
    omega = 1.0 / 10000.0 ** (np.arange(d // 2, dtype=np.float64) / (d / 2.0))
    out = pos.reshape(-1)[:, None] * omega[None, :]
    return np.concatenate([np.sin(out), np.cos(out)], axis=1)


def _pos_embed():
    g = np.meshgrid(np.arange(GRID, dtype=np.float64), np.arange(GRID, dtype=np.float64))
    g = np.stack(g, axis=0)
    emb = np.concatenate([_sincos_1d(DIM // 2, g[0]), _sincos_1d(DIM // 2, g[1])], axis=1)
    emb = np.concatenate([np.zeros((1, DIM)), emb], axis=0)
    return emb.astype(np.float32)  # (Q, DIM)


def _spatial_idx():
    ys, xs = np.meshgrid(np.arange(GRID), np.arange(GRID), indexing='ij')
    coords = np.stack([ys.ravel(), xs.ravel()], axis=1).astype(np.float64)
    d = coords[:, None, :] - coords[None, :, :]
    dis = np.round(np.sqrt((d ** 2).sum(-1)))
    safe = np.maximum(dis, ALPHA)
    far = np.minimum(np.round(ALPHA + np.log(safe / ALPHA) / np.log(GAMMA / ALPHA) * (BETA - ALPHA)), BETA)
    return np.where(dis <= ALPHA, np.round(dis), far).astype(np.int64)  # (P,P)[qs,js]


def band_windows():
    out = []
    for js0, sz in zip(JT_STARTS, JT_SIZES):
        lo = max(0, js0 - BAND)
        hi = min(P, js0 + sz - 1 + BAND + 1)
        out.append((js0, sz, lo, hi))
    return out


def _packed_masks():
    idx = _spatial_idx()
    masks = []
    for u in range(N_LOCAL):
        per_tile = []
        for js0, sz, lo, hi in band_windows():
            per_tile.append((idx[lo:hi, js0:js0 + sz].T == u).astype(DT_E_NP))
        masks.append(per_tile)
    return masks


def _bc(ap0, ap1):
    """Broadcast ap1 against ap0 (stride-0 expansion of size-1 dims)."""
    a, b = bass.broadcast_tensor_aps(ap0, ap1)
    return a, b


# ---------------- device program ----------------
def build_nc():
    nc = bacc.Bacc("TRN2", target_bir_lowering=False, debug=False, num_devices=8)

    xT_d = nc.dram_tensor("xT", [DIM, NKEY], DT_E, kind="ExternalInput")
    wkT_d = nc.dram_tensor("wkT", [DIM, CH], DT_E, kind="ExternalInput")
    wvT_d = nc.dram_tensor("wvT", [DIM, CH], DT_E, kind="ExternalInput")
    projWT_d = nc.dram_tensor("projWT", [CH, DIM], DT_E, kind="ExternalInput")
    posT_d = nc.dram_tensor("posT", [CH, Q], F32, kind="ExternalInput")
    ql_d = nc.dram_tensor("ql", [128, 3], F32, kind="ExternalInput")
    rpeT_d = nc.dram_tensor("rpeT", [HD, 6], F32, kind="ExternalInput")
    wins = band_windows()
    mask_d = [[nc.dram_tensor(f"m{u}_{t}", [sz, hi - lo], DT_E, kind="ExternalInput")
               for t, (js0, sz, lo, hi) in enumerate(wins)] for u in range(N_LOCAL)]
    maskN_d = [nc.dram_tensor(f"mn_{t}", [sz, hi - lo], DT_E, kind="ExternalInput")
               for t, (js0, sz, lo, hi) in enumerate(wins)]
    out_d = nc.dram_tensor("out", [Q, DIM], F32, kind="ExternalOutput")
    dbg = {}
    if DEBUG_TAPS:
        for nm, shp, dt in [("dbg_kT", [128, NKEY], DT_E), ("dbg_v", [128, NH * 65], DT_E),
                            ("dbg_fpb", [128, Q], DT_E), ("dbg_e", [128, Q], DT_E),
                            ("dbg_acc", [128, 280], DT_E), ("dbg_dm", [128, 280], DT_E),
                            ("dbg_hid", [128, Q], DT_E), ("dbg_rbs", [64, Q], F32),
                            ("dbg_qT", [128, Q], F32)]:
            dbg[nm] = nc.dram_tensor(nm, shp, dt, kind="ExternalOutput")

    with tile.TileContext(nc) as tc:
        # ---- persistent SBUF tiles ----
        pers = tc.alloc_tile_pool(name="pers", bufs=1)

        def ptile(shape, dt, nm):
            return pers.tile(shape, dt, name=nm, tag=nm)

        xT = [ptile([128, NKEY], DT_E, f"xT{t}") for t in range(6)]
        wkT = [ptile([128, CH], DT_E, f"wkT{t}") for t in range(6)]
        wvT = [ptile([128, CH], DT_E, f"wvT{t}") for t in range(6)]
        projWT = [ptile([128, DIM], DT_E, f"pW{t}") for t in range(3)]
        qT = [ptile([128, Q], F32, f"qT{t}") for t in range(3)]
        ql = ptile([128, 3], F32, "ql_s")
        rpeT = ptile([128, 6], F32, "rpeT_s")  # rpe_table.T duplicated in both halves
        kT = [ptile([128, NKEY], DT_E, f"kT{t}") for t in range(3)]
        # v tokens, per key-tile, 6 heads x (64 chans + ones col)
        # jt 0 = cls key; jt 1 + m*5 + t = modality m, spatial tile t
        jt_sizes = [1] + [sz for m in range(NM) for sz in JT_SIZES]
        v_aug = [ptile([max(sz, 1), NH * 65], DT_E, f"vA{j}")
                 for j, sz in enumerate(jt_sizes)]
        masks = [[ptile([sz, hi - lo], DT_E, f"ms{u}_{t}")
                  for t, (js0, sz, lo, hi) in enumerate(wins)] for u in range(N_LOCAL)]
        maskN = [ptile([sz, hi - lo], DT_E, f"mn_s{t}")
                 for t, (js0, sz, lo, hi) in enumerate(wins)]
        hidT = [ptile([128, Q], DT_E, f"hidT{t}") for t in range(3)]

        # ---- DMAs in ----
        for t in range(6):
            nc.sync.dma_start(xT[t][:, :], xT_d[128 * t:128 * t + 128, :])
            nc.sync.dma_start(wkT[t][:, :], wkT_d[128 * t:128 * t + 128, :])
            nc.sync.dma_start(wvT[t][:, :], wvT_d[128 * t:128 * t + 128, :])
        for t in range(3):
            nc.sync.dma_start(qT[t][:, :], posT_d[128 * t:128 * t + 128, :])
        for t in range(3):
            nc.sync.dma_start(projWT[t][:, :], projWT_d[128 * t:128 * t + 128, :])
        nc.sync.dma_start(ql[:, :], ql_d[:, :])
        nc.sync.dma_start(rpeT[0:HD, :], rpeT_d[:, :])
        nc.sync.dma_start(rpeT[HD:128, :], rpeT_d[:, :])
        for u in range(N_LOCAL):
            for t in range(5):
                nc.sync.dma_start(masks[u][t][:, :], mask_d[u][t][:, :])
        for t in range(5):
            nc.sync.dma_start(maskN[t][:, :], maskN_d[t][:, :])

        # ---- PSUM pools: 6x 1-bank rotating slots + one 2-bank accumulator ----
        stp = tc.alloc_tile_pool(name="stp", bufs=4, space="PSUM")
        otp = tc.alloc_tile_pool(name="otp", bufs=1, space="PSUM")
        sb = tc.alloc_tile_pool(name="sb", bufs=3)
        bandp = tc.alloc_tile_pool(name="bandp", bufs=3)

        # ---- q = pos + q_learned (in place on qT tiles), then bf16 copy ----
        qTb = [ptile([128, Q], DT_E, f"qTb{t}") for t in range(3)]
        for t in range(3):
            nc.vector.tensor_scalar_add(qT[t][:, :], qT[t][:, :], ql[:, t:t + 1])
            nc.vector.tensor_copy(qTb[t][:, :], qT[t][:, :])

        # ---- replicated-difference stationaries: rep_u[d, m] = rpe[u,d]-rpe[3,d] ----
        # PE broadcasts across m for free: (rep_u.T @ q_h)[m, i] = tmp_u[i]-tmp_3[i]
        ones128 = ptile([128, 128], F32, "ones128")
        nc.vector.memset(ones128[:, :], 1.0)
        UREP = [0, 1, 2, 5]
        reps = [ptile([128, 128], DT_E, f"rep{u}") for u in UREP]
        diffs = ptile([128, 4], F32, "diffs")
        for du, u in enumerate(UREP):
            nc.vector.tensor_tensor(diffs[:, du:du + 1], rpeT[:, u:u + 1],
                                    rpeT[:, 3:4], OP.subtract)
            nc.vector.tensor_scalar_mul(reps[du][:, :], ones128[:, :],
                                        diffs[:, du:du + 1])

        # ---- kT = (wk x)^T in (chan, key) layout ----
        KB = [(0, 512), (512, 1024), (1024, 1536), (1536, 2048), (2048, NKEY)]
        for ct in range(3):
            for (j0, j1) in KB:
                ps = stp.tile([128, j1 - j0], F32, tag="st", name=f"kps{ct}_{j0}")
                for dt in range(6):
                    nc.tensor.matmul(ps[:, :], wkT[dt][:, 128 * ct:128 * ct + 128],
                                     xT[dt][:, j0:j1], start=(dt == 0), stop=(dt == 5))
                nc.vector.tensor_copy(kT[ct][:, j0:j1], ps[:, :])

        # ---- v in (key, chan) layout, strided per head + ones column ----
        jt_ranges = [(0, 1)] + [(1 + m * P + js0, 1 + m * P + js0 + sz)
                                for m in range(NM) for js0, sz in zip(JT_STARTS, JT_SIZES)]
        for j, (k0, k1) in enumerate(jt_ranges):
            sz = k1 - k0
            ps = stp.tile([max(sz, 1), CH], F32, tag="st", name=f"vps{j}")
            for dt in range(6):
                nc.tensor.matmul(ps[:sz, :], xT[dt][:, k0:k1], wvT[dt][:, :],
                                 start=(dt == 0), stop=(dt == 5))
            v3 = v_aug[j][:sz, :].rearrange("p (h c) -> p h c", c=65)
            nc.vector.tensor_copy(v3[:, :, 0:64],
                                  ps[:sz, :].rearrange("p (h c) -> p h c", c=64))
            nc.vector.memset(v3[:, :, 64:65], 1.0)

        if DEBUG_TAPS:
            nc.sync.dma_start(dbg["dbg_kT"][:, :], kT[0][:, :])
            nc.sync.dma_start(dbg["dbg_v"][:, :], v_aug[3][:, :])
            nc.sync.dma_start(dbg["dbg_qT"][:, :], qT[0][:, :])

        # ---- attention, software-pipelined across heads ----
        # Head h's QK+exp matmuls are interleaved in the PE stream with head
        # h-1's V-side (term) matmuls, so the PE never idles waiting on the
        # ACT exp chain.
        NJT = len(jt_ranges)

        def emit_fpb(hl, qh, base):
            fpb = []
            for du, u in enumerate(UREP):
                fb = sb.tile([128, Q], DT_E, tag="fpb", bufs=10, name=f"fpb{hl}_{u}")
                for bi, (i0, i1) in enumerate(IB):
                    tb = stp.tile([128, i1 - i0], F32, tag="st", name=f"tb{hl}_{u}_{bi}")
                    nc.tensor.matmul(tb[:, :], reps[du][base:base + 64, :],
                                     qh[:, i0:i1], start=True, stop=True)
                    nc.scalar.activation(fb[:, i0:i1], tb[:, :], AF.Exp)
                fpb.append(fb)
            return fpb

        def emit_qk(hl, qh, kh, jt):
            k0, k1 = jt_ranges[jt]
            sz = k1 - k0
            e = sb.tile([128, Q], DT_E, tag="eT", bufs=46, name=f"e{hl}_{jt}")
            for bi, (i0, i1) in enumerate(IB):
                st = stp.tile([128, i1 - i0], F32, tag="st", name=f"st{hl}_{jt}_{bi}")
                nc.tensor.matmul(st[:sz, :], kh[:, k0:k1], qh[:, i0:i1],
                                 start=True, stop=True)
                nc.scalar.activation(e[:sz, i0:i1], st[:sz, :], AF.Exp, scale=SCALE)
            return e

        def emit_acc(hl, fpb, t):
            js0, sz, lo, hi = wins[t]
            W = hi - lo
            acc = bandp.tile([128, 280], DT_E, tag="acc", bufs=12, name=f"acc{hl}_{t}")
            scr = bandp.tile([128, 280], DT_E, tag="scr", name=f"scr{hl}_{t}")
            nc.vector.tensor_tensor(acc[:sz, :W], masks[0][t][:, :],
                                    fpb[0][:sz, 1 + lo:1 + hi], OP.mult)
            for u in range(1, N_LOCAL):
                nc.vector.tensor_tensor(scr[:sz, :W], masks[u][t][:, :],
                                        fpb[u][:sz, 1 + lo:1 + hi], OP.mult)
                nc.vector.tensor_tensor(acc[:sz, :W], acc[:sz, :W],
                                        scr[:sz, :W], OP.add)
            nc.vector.tensor_tensor(acc[:sz, :W], acc[:sz, :W],
                                    maskN[t][:, :], OP.add)
            return acc

        def terms_gen(hl, fpb, es, accs):
            """Yields between chunks so head hl+1's QK matmuls interleave."""
            ot = otp.tile([65, Q], F32, tag="ot", bufs=2, name=f"ot{hl}")

            def term1(jt, first=False, last=False):
                k0, k1 = jt_ranges[jt]
                sz = k1 - k0
                for bi, (i0, i1) in enumerate(IB):
                    nc.tensor.matmul(ot[:, i0:i1],
                                     v_aug[jt][:sz, 65 * hl:65 * hl + 65],
                                     es[jt][:sz, i0:i1], start=first, stop=last)

            # cls key: term1 + multiplicative cls-row correction (bucket 5)
            term1(0, first=True)
            dcls = bandp.tile([1, Q], DT_E, tag="dcls", name=f"dcls{hl}")
            nc.vector.tensor_tensor(dcls[:, 1:Q], es[0][0:1, 1:Q],
                                    fpb[3][0:1, 1:Q], OP.mult)
            nc.vector.tensor_tensor(dcls[:, 1:Q], dcls[:, 1:Q],
                                    es[0][0:1, 1:Q], OP.subtract)
            for (i0, i1) in IB:
                lo2 = max(i0, 1)
                nc.tensor.matmul(ot[:, lo2:i1], v_aug[0][0:1, 65 * hl:65 * hl + 65],
                                 dcls[:, lo2:i1], start=False, stop=False)
            yield
            for t in range(5):
                for m in range(NM):
                    term1(1 + m * 5 + t)
                yield
            for t, (js0, sz, lo, hi) in enumerate(wins):
                W = hi - lo
                for m in range(NM):
                    jt = 1 + m * 5 + t
                    e = es[jt]
                    last = (t == 4 and m == NM - 1)
                    dm = bandp.tile([128, 280], DT_E, tag="dm", bufs=4,
                                    name=f"dm{hl}_{jt}")
                    nc.vector.tensor_tensor(dm[:sz, :W], accs[t][:sz, :W],
                                            e[:sz, 1 + lo:1 + hi], OP.mult)
                    # term-2 pieces split at the query-bank boundary (512)
                    pieces = []
                    c0, c1 = 1 + lo, 1 + hi
                    if c0 < 512:
                        pieces.append((c0, min(512, c1)))
                    if c1 > 512:
                        pieces.append((max(512, c0), c1))
                    for (p0, p1) in pieces:
                        # last (t, m): one piece per query bank -> stop both groups
                        nc.tensor.matmul(
                            ot[:, p0:p1], v_aug[jt][:sz, 65 * hl:65 * hl + 65],
                            dm[:sz, p0 - c0:p1 - c0],
                            start=False, stop=last)
                    yield
            # normalize: hid = num * (1/den); PE broadcasts recip across rows
            rc = sb.tile([1, Q], F32, tag="rc", name=f"rc{hl}")
            den = sb.tile([1, Q], F32, tag="den", name=f"den{hl}")
            nc.vector.tensor_copy(den[:, :], ot[64:65, :])
            nc.vector.reciprocal_approx_fast(rc[:, :], den[:, :])
            rbs = sb.tile([64, Q], F32, tag="rbs", name=f"rbs{hl}")
            for bi, (i0, i1) in enumerate(IB):
                rb = stp.tile([64, i1 - i0], F32, tag="st", name=f"rb{hl}_{bi}")
                nc.tensor.matmul(rb[:, :], ones128[0:1, 0:64],
                                 rc[:, i0:i1], start=True, stop=True)
                nc.vector.tensor_copy(rbs[:, i0:i1], rb[:, :])
            nc.vector.tensor_tensor(
                hidT[hl // 2][64 * (hl % 2):64 * (hl % 2) + 64, :],
                ot[0:64, :], rbs[:, :], OP.mult)
            yield

        prev_gen = None
        for hl in range(NH):
            qh = qTb[hl // 2][64 * (hl % 2):64 * (hl % 2) + 64, :]
            kh = kT[hl // 2][64 * (hl % 2):64 * (hl % 2) + 64, :]
            base = 64 * (hl % 2)
            fpb = emit_fpb(hl, qh, base)
            es = []
            for jt in range(NJT):
                es.append(emit_qk(hl, qh, kh, jt))
                if prev_gen is not None:
                    next(prev_gen, None)
                    next(prev_gen, None)
            accs = [emit_acc(hl, fpb, t) for t in range(5)]
            if prev_gen is not None:
                for _ in prev_gen:
                    pass
            prev_gen = terms_gen(hl, fpb, es, accs)
        for _ in prev_gen:
            pass

        if DEBUG_TAPS:
            nc.sync.dma_start(dbg["dbg_hid"][:, :], hidT[0][:, :])

        # ---- partial output projection: out = hidT^T @ projWT ----
        OB = [(0, 512), (512, DIM)]
        ITS = [(0, 128), (128, 256), (256, 384), (384, 512), (512, Q)]
        for (r0, r1) in ITS:
            szr = r1 - r0
            ob = sb.tile([128, DIM], F32, tag="ob", name=f"ob{r0}")
            for (c0, c1) in OB:
                ps = stp.tile([128, 512], F32, tag="st", name=f"ops{r0}_{c0}")
                for ct in range(3):
                    nc.tensor.matmul(ps[:szr, :c1 - c0], hidT[ct][:, r0:r1],
                                     projWT[ct][:, c0:c1], start=(ct == 0), stop=(ct == 2))
                nc.vector.tensor_copy(ob[:szr, c0:c1], ps[:szr, :c1 - c0])
            nc.sync.dma_start(out_d[r0:r1, :], ob[:szr, :])

        for pool in (bandp, sb, otp, stp, pers):
            pool.release()

    nc.compile()
    return nc


wins = band_windows()

_NC = None


def _get_nc():
    global _NC
    if _NC is None:
        _NC = build_nc()
    return _NC


def make_in_maps(x, wk, wv, proj_w, q_learned, rpe_table):
    pos = _pos_embed()
    masks = _packed_masks()
    rpeT = np.ascontiguousarray(rpe_table.T).astype(np.float32)
    common = {"rpeT": rpeT}
    for u in range(N_LOCAL):
        for t in range(5):
            common[f"m{u}_{t}"] = np.ascontiguousarray(masks[u][t])
    for t in range(5):
        mn = -(masks[0][t].astype(np.float32) + masks[1][t].astype(np.float32)
               + masks[2][t].astype(np.float32))
        common[f"mn_{t}"] = np.ascontiguousarray(mn.astype(DT_E_NP))
    in_maps = []
    for c in range(8):
        b, hh = c // 2, c % 2
        m = dict(common)
        m["xT"] = np.ascontiguousarray(x[b].T).astype(DT_E_NP)
        m["posT"] = np.ascontiguousarray(pos.T[CH * hh:CH * hh + CH]).astype(np.float32)
        m["ql"] = np.ascontiguousarray(
            q_learned[CH * hh:CH * hh + CH].reshape(3, 128).T).astype(np.float32)
        m["wkT"] = np.ascontiguousarray(wk[CH * hh:CH * hh + CH].T).astype(DT_E_NP)
        m["wvT"] = np.ascontiguousarray(wv[CH * hh:CH * hh + CH].T).astype(DT_E_NP)
        m["projWT"] = np.ascontiguousarray(proj_w[:, CH * hh:CH * hh + CH].T).astype(DT_E_NP)
        in_maps.append(m)
    return in_maps


def kernel(x, wk, wv, proj_w, proj_b, q_learned, rpe_table, _results_hook=None):
    x = np.asarray(x, dtype=np.float32)
    nc = _get_nc()
    in_maps = make_in_maps(x, np.asarray(wk), np.asarray(wv), np.asarray(proj_w),
                           np.asarray(q_learned), np.asarray(rpe_table))
    res = run_bass_kernel_spmd(nc, in_maps, core_ids=list(range(8)))
    if _results_hook is not None:
        _results_hook(res)
    out = np.zeros((B, Q, DIM), np.float32)
    for c in range(8):
        out[c // 2] += np.asarray(res.results[c]["out"], dtype=np.float32)
    out += np.asarray(proj_b, dtype=np.float32)[None, None, :]
    return out


# revision 31
# speedup vs baseline: 1.0419x; 1.0419x over previous
"""Trainium2 Bass kernel for nn_CrossRPEAttention (B=4, H=12, DIM=768, Q=577, N=2305).

Sharding: 8 cores = batch(4) x head-half(2). Each core computes, for its
(b, hh): k/v projections for its 384 channels, cross-attention with iRPE
contextual bias for 6 heads, and a partial output projection. Host sums the
two head-half partials per batch and adds proj_b.

Attention is computed in S^T layout (keys on partitions, queries on free dim)
so no on-device transposes are needed anywhere:
  - logits^T tile per key-tile via PE (contraction HD=64)
  - softmax shift-invariance: bias is taken relative to the far-field bucket
    (u=3), which covers all rounded-distances >= 4, so the remaining bias
    correction is multiplicative and local: |dy|,|dx| <= 3 (band +-75 in
    raveled index)
  - unnorm = exp(scale*S^T) * (1 + sum_u F'_u[i] * M_u[js,i]) applied as
    banded rank-full masks on DVE, folded into extra accumulating matmuls
  - denominator comes free from a ones-column appended to v
  - the cls-query column (i=0) bias is a per-column constant -> cancels in
    softmax and is skipped entirely
"""
import sys
import numpy as np

sys.path.insert(0, "/opt/trn_rl_repo")

import concourse.bass as bass
import concourse.bacc as bacc
import concourse.mybir as mybir
import concourse.tile as tile
from concourse.bass_utils import run_bass_kernel_spmd

try:
    import ml_dtypes
    BF16_NP = ml_dtypes.bfloat16
except ImportError:  # pragma: no cover
    BF16_NP = np.float32

F32 = mybir.dt.float32
BF16 = mybir.dt.bfloat16
OP = mybir.AluOpType
AF = mybir.ActivationFunctionType

# ---------------- problem constants ----------------
B, DIM, H, HD, GRID = 4, 768, 12, 64, 24
P = GRID * GRID            # 576 spatial patches
NM = 4                     # modalities
Q = 1 + P                  # 577 queries
NKEY = 1 + NM * P          # 2305 keys
SCALE = HD ** -0.5
ALPHA, BETA, GAMMA = 1.9, 3.8, 15.2
BAND = 75
JT_STARTS = [0, 128, 256, 384, 512]
JT_SIZES = [128, 128, 128, 128, 64]
N_LOCAL = 3                # local correction buckets u = 0,1,2
BASE_BUCKET = 3            # far-field bucket (rounded dist >= 4)
CLS_BUCKET = 5
NH = 6                     # heads per core
CH = NH * HD               # 384 channels per core
IB = [(0, 512), (512, Q)]  # query (free dim) bank splits

# attention-path dtype (flip to F32/np.float32 if precision demands)
DT_E = BF16
DT_E_NP = BF16_NP
DEBUG_TAPS = False


# ---------------- host-side constants ----------------
def _sincos_1d(d, pos):
    omega = 1.0 / 10000.0 ** (np.arange(d // 2, dtype=np.float64) / (d / 2.0))
    out = pos.reshape(-1)[:, None] * omega[None, :]
    return np.concatenate([np.sin(out), np.cos(out)], axis=1)


def _pos_embed():
    g = np.meshgrid(np.arange(GRID, dtype=np.float64), np.arange(GRID, dtype=np.float64))
    g = np.stack(g, axis=0)
    emb = np.concatenate([_sincos_1d(DIM // 2, g[0]), _sincos_1d(DIM // 2, g[1])], axis=1)
    emb = np.concatenate([np.zeros((1, DIM)), emb], axis=0)
    return emb.astype(np.float32)  # (Q, DIM)


def _spatial_idx():
    ys, xs = np.meshgrid(np.arange(GRID), np.arange(GRID), indexing='ij')
    coords = np.stack([ys.ravel(), xs.ravel()], axis=1).astype(np.float64)
    d = coords[:, None, :] - coords[None, :, :]
    dis = np.round(np.sqrt((d ** 2).sum(-1)))
    safe = np.maximum(dis, ALPHA)
    far = np.minimum(np.round(ALPHA + np.log(safe / ALPHA) / np.log(GAMMA / ALPHA) * (BETA - ALPHA)), BETA)
    return np.where(dis <= ALPHA, np.round(dis), far).astype(np.int64)  # (P,P)[qs,js]


def band_windows():
    out = []
    for js0, sz in zip(JT_STARTS, JT_SIZES):
        lo = max(0, js0 - BAND)
        hi = min(P, js0 + sz - 1 + BAND + 1)
        out.append((js0, sz, lo, hi))
    return out


def _packed_masks():
    idx = _spatial_idx()
    masks = []
    for u in range(N_LOCAL):
        per_tile = []
        for js0, sz, lo, hi in band_windows():
            per_tile.append((idx[lo:hi, js0:js0 + sz].T == u).astype(DT_E_NP))
        masks.append(per_tile)
    return masks


def _bc(ap0, ap1):
    """Broadcast ap1 against ap0 (stride-0 expansion of size-1 dims)."""
    a, b = bass.broadcast_tensor_aps(ap0, ap1)
    return a, b


# ---------------- device program ----------------
def build_nc():
    nc = bacc.Bacc("TRN2", target_bir_lowering=False, debug=False, num_devices=8)

    xT_d = nc.dram_tensor("xT", [DIM, NKEY], DT_E, kind="ExternalInput")
    wkT_d = nc.dram_tensor("wkT", [DIM, CH], DT_E, kind="ExternalInput")
    wvT_d = nc.dram_tensor("wvT", [DIM, CH], DT_E, kind="ExternalInput")
    projWT_d = nc.dram_tensor("projWT", [CH, DIM], DT_E, kind="ExternalInput")
    posT_d = nc.dram_tensor("posT", [CH, Q], F32, kind="ExternalInput")
    ql_d = nc.dram_tensor("ql", [128, 3], F32, kind="ExternalInput")
    rpeT_d = nc.dram_tensor("rpeT", [HD, 6], F32, kind="ExternalInput")
    wins = band_windows()
    mask_d = [[nc.dram_tensor(f"m{u}_{t}", [sz, hi - lo], DT_E, kind="ExternalInput")
               for t, (js0, sz, lo, hi) in enumerate(wins)] for u in range(N_LOCAL)]
    maskN_d = [nc.dram_tensor(f"mn_{t}", [sz, hi - lo], DT_E, kind="ExternalInput")
               for t, (js0, sz, lo, hi) in enumerate(wins)]
    out_d = nc.dram_tensor("out", [Q, DIM], F32, kind="ExternalOutput")
    dbg = {}
    if DEBUG_TAPS:
        for nm, shp, dt in [("dbg_kT", [128, NKEY], DT_E), ("dbg_v", [128, NH * 65], DT_E),
                            ("dbg_fpb", [128, Q], DT_E), ("dbg_e", [128, Q], DT_E),
                            ("dbg_acc", [128, 280], DT_E), ("dbg_dm", [128, 280], DT_E),
                            ("dbg_hid", [128, Q], DT_E), ("dbg_rbs", [64, Q], F32),
                            ("dbg_qT", [128, Q], F32)]:
            dbg[nm] = nc.dram_tensor(nm, shp, dt, kind="ExternalOutput")

    with tile.TileContext(nc) as tc:
        # ---- persistent SBUF tiles ----
        pers = tc.alloc_tile_pool(name="pers", bufs=1)

        def ptile(shape, dt, nm):
            return pers.tile(shape, dt, name=nm, tag=nm)

        xT = [ptile([128, NKEY], DT_E, f"xT{t}") for t in range(6)]
        wkT = [ptile([128, CH], DT_E, f"wkT{t}") for t in range(6)]
        wvT = [ptile([128, CH], DT_E, f"wvT{t}") for t in range(6)]
        projWT = [ptile([128, DIM], DT_E, f"pW{t}") for t in range(3)]
        qT = [ptile([128, Q], F32, f"qT{t}") for t in range(3)]
        ql = ptile([128, 3], F32, "ql_s")
        rpeT = ptile([128, 6], F32, "rpeT_s")  # rpe_table.T duplicated in both halves
        kT = [ptile([128, NKEY], DT_E, f"kT{t}") for t in range(3)]
        # v tokens, per key-tile, 6 heads x (64 chans + ones col)
        # jt 0 = cls key; jt 1 + m*5 + t = modality m, spatial tile t
        jt_sizes = [1] + [sz for m in range(NM) for sz in JT_SIZES]
        v_aug = [ptile([max(sz, 1), NH * 65], DT_E, f"vA{j}")
                 for j, sz in enumerate(jt_sizes)]
        masks = [[ptile([sz, hi - lo], DT_E, f"ms{u}_{t}")
                  for t, (js0, sz, lo, hi) in enumerate(wins)] for u in range(N_LOCAL)]
        maskN = [ptile([sz, hi - lo], DT_E, f"mn_s{t}")
                 for t, (js0, sz, lo, hi) in enumerate(wins)]
        hidT = [ptile([128, Q], DT_E, f"hidT{t}") for t in range(3)]

        # ---- DMAs in ----
        for t in range(6):
            nc.sync.dma_start(xT[t][:, :], xT_d[128 * t:128 * t + 128, :])
            nc.sync.dma_start(wkT[t][:, :], wkT_d[128 * t:128 * t + 128, :])
            nc.sync.dma_start(wvT[t][:, :], wvT_d[128 * t:128 * t + 128, :])
        for t in range(3):
            nc.sync.dma_start(qT[t][:, :], posT_d[128 * t:128 * t + 128, :])
        for t in range(3):
            nc.sync.dma_start(projWT[t][:, :], projWT_d[128 * t:128 * t + 128, :])
        nc.sync.dma_start(ql[:, :], ql_d[:, :])
        nc.sync.dma_start(rpeT[0:HD, :], rpeT_d[:, :])
        nc.sync.dma_start(rpeT[HD:128, :], rpeT_d[:, :])
        for u in range(N_LOCAL):
            for t in range(5):
                nc.sync.dma_start(masks[u][t][:, :], mask_d[u][t][:, :])
        for t in range(5):
            nc.sync.dma_start(maskN[t][:, :], maskN_d[t][:, :])

        # ---- PSUM pools: 6x 1-bank rotating slots + one 2-bank accumulator ----
        stp = tc.alloc_tile_pool(name="stp", bufs=4, space="PSUM")
        otp = tc.alloc_tile_pool(name="otp", bufs=1, space="PSUM")
        sb = tc.alloc_tile_pool(name="sb", bufs=3)
        bandp = tc.alloc_tile_pool(name="bandp", bufs=3)

        # ---- q = pos + q_learned (in place on qT tiles), then bf16 copy ----
        qTb = [ptile([128, Q], DT_E, f"qTb{t}") for t in range(3)]
        for t in range(3):
            nc.vector.tensor_scalar_add(qT[t][:, :], qT[t][:, :], ql[:, t:t + 1])
            nc.vector.tensor_copy(qTb[t][:, :], qT[t][:, :])

        # ---- replicated-difference stationaries: rep_u[d, m] = rpe[u,d]-rpe[3,d] ----
        # PE broadcasts across m for free: (rep_u.T @ q_h)[m, i] = tmp_u[i]-tmp_3[i]
        ones128 = ptile([128, 128], F32, "ones128")
        nc.vector.memset(ones128[:, :], 1.0)
        UREP = [0, 1, 2, 5]
        reps = [ptile([128, 128], DT_E, f"rep{u}") for u in UREP]
        diffs = ptile([128, 4], F32, "diffs")
        for du, u in enumerate(UREP):
            nc.vector.tensor_tensor(diffs[:, du:du + 1], rpeT[:, u:u + 1],
                                    rpeT[:, 3:4], OP.subtract)
            nc.vector.tensor_scalar_mul(reps[du][:, :], ones128[:, :],
                                        diffs[:, du:du + 1])

        # ---- kT = (wk x)^T in (chan, key) layout ----
        KB = [(0, 512), (512, 1024), (1024, 1536), (1536, 2048), (2048, NKEY)]
        for ct in range(3):
            for (j0, j1) in KB:
                ps = stp.tile([128, j1 - j0], F32, tag="st", name=f"kps{ct}_{j0}")
                for dt in range(6):
                    nc.tensor.matmul(ps[:, :], wkT[dt][:, 128 * ct:128 * ct + 128],
                                     xT[dt][:, j0:j1], start=(dt == 0), stop=(dt == 5))
                nc.vector.tensor_copy(kT[ct][:, j0:j1], ps[:, :])

        # ---- v in (key, chan) layout, strided per head + ones column ----
        jt_ranges = [(0, 1)] + [(1 + m * P + js0, 1 + m * P + js0 + sz)
                                for m in range(NM) for js0, sz in zip(JT_STARTS, JT_SIZES)]
        for j, (k0, k1) in enumerate(jt_ranges):
            sz = k1 - k0
            ps = stp.tile([max(sz, 1), CH], F32, tag="st", name=f"vps{j}")
            for dt in range(6):
                nc.tensor.matmul(ps[:sz, :], xT[dt][:, k0:k1], wvT[dt][:, :],
                                 start=(dt == 0), stop=(dt == 5))
            v3 = v_aug[j][:sz, :].rearrange("p (h c) -> p h c", c=65)
            nc.vector.tensor_copy(v3[:, :, 0:64],
                                  ps[:sz, :].rearrange("p (h c) -> p h c", c=64))
            nc.vector.memset(v3[:, :, 64:65], 1.0)

        if DEBUG_TAPS:
            nc.sync.dma_start(dbg["dbg_kT"][:, :], kT[0][:, :])
            nc.sync.dma_start(dbg["dbg_v"][:, :], v_aug[3][:, :])
            nc.sync.dma_start(dbg["dbg_qT"][:, :], qT[0][:, :])

        # ---- attention, software-pipelined across heads ----
        # Head h's QK+exp matmuls are interleaved in the PE stream with head
        # h-1's V-side (term) matmuls, so the PE never idles waiting on the
        # ACT exp chain.
        NJT = len(jt_ranges)

        def emit_fpb(hl, qh, base):
            fpb = []
            for du, u in enumerate(UREP):
                fb = sb.tile([128, Q], DT_E, tag="fpb", bufs=10, name=f"fpb{hl}_{u}")
                for bi, (i0, i1) in enumerate(IB):
                    tb = stp.tile([128, i1 - i0], F32, tag="st", name=f"tb{hl}_{u}_{bi}")
                    nc.tensor.matmul(tb[:, :], reps[du][base:base + 64, :],
                                     qh[:, i0:i1], start=True, stop=True)
                    nc.scalar.activation(fb[:, i0:i1], tb[:, :], AF.Exp)
                fpb.append(fb)
            return fpb

        def emit_qk(hl, qh, kh, jt):
            k0, k1 = jt_ranges[jt]
            sz = k1 - k0
            e = sb.tile([128, Q], DT_E, tag="eT", bufs=46, name=f"e{hl}_{jt}")
            for bi, (i0, i1) in enumerate(IB):
                st = stp.tile([128, i1 - i0], F32, tag="st", name=f"st{hl}_{jt}_{bi}")
                nc.tensor.matmul(st[:sz, :], kh[:, k0:k1], qh[:, i0:i1],
                                 start=True, stop=True)
                nc.scalar.activation(e[:sz, i0:i1], st[:sz, :], AF.Exp, scale=SCALE)
            return e

        def emit_acc(hl, fpb, t):
            js0, sz, lo, hi = wins[t]
            W = hi - lo
            acc = bandp.tile([128, 280], DT_E, tag="acc", bufs=12, name=f"acc{hl}_{t}")
            scr = bandp.tile([128, 280], DT_E, tag="scr", name=f"scr{hl}_{t}")
            nc.vector.tensor_tensor(acc[:sz, :W], masks[0][t][:, :],
                                    fpb[0][:sz, 1 + lo:1 + hi], OP.mult)
            for u in range(1, N_LOCAL):
                nc.vector.tensor_tensor(scr[:sz, :W], masks[u][t][:, :],
                                        fpb[u][:sz, 1 + lo:1 + hi], OP.mult)
                nc.vector.tensor_tensor(acc[:sz, :W], acc[:sz, :W],
                                        scr[:sz, :W], OP.add)
            nc.vector.tensor_tensor(acc[:sz, :W], acc[:sz, :W],
                                    maskN[t][:, :], OP.add)
            return acc

        def terms_gen(hl, fpb, es, accs):
            """One jt consumed per next(); es[k] must exist before step k."""
            ot = otp.tile([65, Q], F32, tag="ot", bufs=2, name=f"ot{hl}")

            def term1(jt, first=False, last=False):
                k0, k1 = jt_ranges[jt]
                sz = k1 - k0
                for bi, (i0, i1) in enumerate(IB):
                    nc.tensor.matmul(ot[:, i0:i1],
                                     v_aug[jt][:sz, 65 * hl:65 * hl + 65],
                                     es[jt][:sz, i0:i1], start=first, stop=last)

            # cls key: term1 + multiplicative cls-row correction (bucket 5)
            term1(0, first=True)
            dcls = bandp.tile([1, Q], DT_E, tag="dcls", name=f"dcls{hl}")
            nc.vector.tensor_tensor(dcls[:, 1:Q], es[0][0:1, 1:Q],
                                    fpb[3][0:1, 1:Q], OP.mult)
            nc.vector.tensor_tensor(dcls[:, 1:Q], dcls[:, 1:Q],
                                    es[0][0:1, 1:Q], OP.subtract)
            for (i0, i1) in IB:
                lo2 = max(i0, 1)
                nc.tensor.matmul(ot[:, lo2:i1], v_aug[0][0:1, 65 * hl:65 * hl + 65],
                                 dcls[:, lo2:i1], start=False, stop=False)
            yield
            for jt in range(1, NJT):
                t, m = (jt - 1) % 5, (jt - 1) // 5
                js0, sz, lo, hi = wins[t]
                W = hi - lo
                last = (jt == NJT - 1)
                term1(jt, last=False)
                dm = bandp.tile([128, 280], DT_E, tag="dm", bufs=4,
                                name=f"dm{hl}_{jt}")
                nc.vector.tensor_tensor(dm[:sz, :W], accs[t][:sz, :W],
                                        es[jt][:sz, 1 + lo:1 + hi], OP.mult)
                # term-2 pieces split at the query-bank boundary (512)
                pieces = []
                c0, c1 = 1 + lo, 1 + hi
                if c0 < 512:
                    pieces.append((c0, min(512, c1)))
                if c1 > 512:
                    pieces.append((max(512, c0), c1))
                for (p0, p1) in pieces:
                    # jt 20 = (t=4, m=3): one piece per query bank -> stop both
                    nc.tensor.matmul(
                        ot[:, p0:p1], v_aug[jt][:sz, 65 * hl:65 * hl + 65],
                        dm[:sz, p0 - c0:p1 - c0],
                        start=False, stop=last)
                yield
            # normalize: hid = num * (1/den); PE broadcasts recip across rows
            rc = sb.tile([1, Q], F32, tag="rc", name=f"rc{hl}")
            den = sb.tile([1, Q], F32, tag="den", name=f"den{hl}")
            nc.vector.tensor_copy(den[:, :], ot[64:65, :])
            nc.vector.reciprocal_approx_fast(rc[:, :], den[:, :])
            rbs = sb.tile([64, Q], F32, tag="rbs", name=f"rbs{hl}")
            for bi, (i0, i1) in enumerate(IB):
                rb = stp.tile([64, i1 - i0], F32, tag="st", name=f"rb{hl}_{bi}")
                nc.tensor.matmul(rb[:, :], ones128[0:1, 0:64],
                                 rc[:, i0:i1], start=True, stop=True)
                nc.vector.tensor_copy(rbs[:, i0:i1], rb[:, :])
            nc.vector.tensor_tensor(
                hidT[hl // 2][64 * (hl % 2):64 * (hl % 2) + 64, :],
                ot[0:64, :], rbs[:, :], OP.mult)
            yield

        # Head pairs: the pair's QK/FpB matmuls sit in opposite 64-row groups
        # (partition base 0 vs 64), so adjacent-in-stream MMs overlap in the
        # PE array. Terms lag LAG jts behind their head's QKs to fill ACT-paced
        # gaps; leftover term tails drain during the next pair's QK phase.
        LAG = 4
        active = []
        for hp in range(3):
            pair = []
            for hl in (2 * hp, 2 * hp + 1):
                qh = qTb[hl // 2][64 * (hl % 2):64 * (hl % 2) + 64, :]
                kh = kT[hl // 2][64 * (hl % 2):64 * (hl % 2) + 64, :]
                pair.append((hl, qh, kh, 64 * (hl % 2)))
            fpbs = [emit_fpb(hl, qh, base) for (hl, qh, kh, base) in pair]
            accs = [[emit_acc(hl, fb, t) for t in range(5)]
                    for (hl, qh, kh, base), fb in zip(pair, fpbs)]
            ess = [[], []]
            gens = [terms_gen(pair[i][0], fpbs[i], ess[i], accs[i])
                    for i in range(2)]
            for jt in range(NJT):
                for i, (hl, qh, kh, base) in enumerate(pair):
                    ess[i].append(emit_qk(hl, qh, kh, jt))
                if jt >= LAG:
                    for g in gens:
                        next(g, None)
                for g in active:
                    next(g, None)
            active = [g for g in active + gens]
        for g in active:
            for _ in g:
                pass

        if DEBUG_TAPS:
            nc.sync.dma_start(dbg["dbg_hid"][:, :], hidT[0][:, :])

        # ---- partial output projection: out = hidT^T @ projWT ----
        OB = [(0, 512), (512, DIM)]
        ITS = [(0, 128), (128, 256), (256, 384), (384, 512), (512, Q)]
        for (r0, r1) in ITS:
            szr = r1 - r0
            ob = sb.tile([128, DIM], F32, tag="ob", name=f"ob{r0}")
            for (c0, c1) in OB:
                ps = stp.tile([128, 512], F32, tag="st", name=f"ops{r0}_{c0}")
                for ct in range(3):
                    nc.tensor.matmul(ps[:szr, :c1 - c0], hidT[ct][:, r0:r1],
                                     projWT[ct][:, c0:c1], start=(ct == 0), stop=(ct == 2))
                nc.vector.tensor_copy(ob[:szr, c0:c1], ps[:szr, :c1 - c0])
            nc.sync.dma_start(out_d[r0:r1, :], ob[:szr, :])

        for pool in (bandp, sb, otp, stp, pers):
            pool.release()

    nc.compile()
    return nc


wins = band_windows()

_NC = None


def _get_nc():
    global _NC
    if _NC is None:
        _NC = build_nc()
    return _NC


def make_in_maps(x, wk, wv, proj_w, q_learned, rpe_table):
    pos = _pos_embed()
    masks = _packed_masks()
    rpeT = np.ascontiguousarray(rpe_table.T).astype(np.float32)
    common = {"rpeT": rpeT}
    for u in range(N_LOCAL):
        for t in range(5):
            common[f"m{u}_{t}"] = np.ascontiguousarray(masks[u][t])
    for t in range(5):
        mn = -(masks[0][t].astype(np.float32) + masks[1][t].astype(np.float32)
               + masks[2][t].astype(np.float32))
        common[f"mn_{t}"] = np.ascontiguousarray(mn.astype(DT_E_NP))
    in_maps = []
    for c in range(8):
        b, hh = c // 2, c % 2
        m = dict(common)
        m["xT"] = np.ascontiguousarray(x[b].T).astype(DT_E_NP)
        m["posT"] = np.ascontiguousarray(pos.T[CH * hh:CH * hh + CH]).astype(np.float32)
        m["ql"] = np.ascontiguousarray(
            q_learned[CH * hh:CH * hh + CH].reshape(3, 128).T).astype(np.float32)
        m["wkT"] = np.ascontiguousarray(wk[CH * hh:CH * hh + CH].T).astype(DT_E_NP)
        m["wvT"] = np.ascontiguousarray(wv[CH * hh:CH * hh + CH].T).astype(DT_E_NP)
        m["projWT"] = np.ascontiguousarray(proj_w[:, CH * hh:CH * hh + CH].T).astype(DT_E_NP)
        in_maps.append(m)
    return in_maps


def kernel(x, wk, wv, proj_w, proj_b, q_learned, rpe_table, _results_hook=None):
    x = np.asarray(x, dtype=np.float32)
    nc = _get_nc()
    in_maps = make_in_maps(x, np.asarray(wk), np.asarray(wv), np.asarray(proj_w),
                           np.asarray(q_learned), np.asarray(rpe_table))
    res = run_bass_kernel_spmd(nc, in_maps, core_ids=list(range(8)))
    if _results_hook is not None:
        _results_hook(res)
    out = np.zeros((B, Q, DIM), np.float32)
    for c in range(8):
        out[c // 2] += np.asarray(res.results[c]["out"], dtype=np.float32)
    out += np.asarray(proj_b, dtype=np.float32)[None, None, :]
    return out
